# revision 25
# baseline (speedup 1.0000x reference)
"""MoE fusion kernel for Trainium2 (8 NeuronCores, Bass/Tile) — v3.

Problem: concat 4 feature maps -> router (GAP -> linear -> softmax -> top-2)
-> per-expert 1x1x1 conv + BN(eval) + ReLU -> weighted combine + aux loss.

Sharding: spatial dim S=D*H*W=32768 split into 8 chunks of 4096, one per
core; each core processes BOTH batch samples for its chunk (every HBM byte
touched exactly once).  The global-average-pool needs a cross-core
reduction: each core computes partial channel sums for its chunk, a 1KB
AllGather shares the partials, then every core redundantly runs the tiny
router math on-device and computes its chunk of the output.

Key points:
- Only the 2 SELECTED experts are computed.  Top-2 expert indices are
  derived on-device with mask/iota tricks, converted to int32, loaded into
  SP-engine registers (values_load) and used as dynamic DRAM offsets to
  gather the selected weight blocks (and BN-bias columns in the general
  path).  No control flow.
- Matmuls run in float32r (TF32-like, ~2e-4 rel err, 4x fp32 throughput).
  x is stored in SBUF as f32r; pooling reads the same bytes bitcast to
  fp32, so the router sees exact fp32 data.
- Combine uses relu(sw*z) == sw*relu(z) for sw>=0.  Fast path (BN bias
  identically 0, true for eval-mode BN with zero beta/running-mean): slot-1
  weights are pre-scaled by sw1 on DVE, so the epilogue is 2 ops/tile:
  ACT y0=Relu(sw0*z0) and DVE ot=max(z1,0)+y0 (fused scalar_tensor_tensor).
  General path: 3 ops/tile with per-partition scale+bias, ACT/DVE balanced.
"""

import numpy as np

B, C, D, H, W = 2, 32, 32, 32, 32
M = 4
MC = M * C          # 128
S = D * H * W       # 32768
E, O = 4, 128
BN_EPS = 1e-5
NCORES = 8
S_LOC = S // NCORES  # 4096
NT = 1024            # epilogue tile (2 PSUM banks)
N_TILES = S_LOC // NT
TOPK = 2

# fp32 constant bank layout ([128, NCONST] "consts" input)
_WRT = slice(0, 4)                 # [128, 4]  router weights^T / S
_BIAST = slice(4, 8)               # [128, 4]  BN bias^T  (iota_b base = 4)
_ONES2 = slice(8, 9)               # [2, 1]    ones (rows 0:2)
_BSEL = slice(9, 265)              # [2, 256]  batch-row selector masks
_BR2 = slice(265, 269)             # [2, 4]    router bias (broadcast rows)
_ID = slice(269, 397)              # [128,128] identity (PE transpose)
_SELB = slice(397, 399)            # [16, 2]   AllGather row->batch selector
NCONST = 399

# general-path epilogue engine schedule per tile: 'A' DVE-heavy, 'B' ACT-heavy
MODES = ["A", "B", "A", "B"]

TRACE = False
TRACE_KWARGS = {}
LAST_RESULTS = None

_BUILD_CACHE = {}


def _build(fast: bool, repeat: int = 1):
    import concourse.bacc as bacc
    import concourse.bass as bass
    import concourse.mybir as mybir
    import concourse.tile as tile

    f32 = mybir.dt.float32
    f32r = mybir.dt.float32r
    i32 = mybir.dt.int32
    AX = mybir.AxisListType
    ALU = mybir.AluOpType
    AF = mybir.ActivationFunctionType
    SP = mybir.EngineType.SP

    nc = bacc.Bacc("TRN2", target_bir_lowering=False, debug=False,
                   num_devices=NCORES)

    x_d = nc.dram_tensor("x", [B, MC, S_LOC], f32r, kind="ExternalInput").ap()
    wct_d = nc.dram_tensor("wct", [MC, E * O], f32, kind="ExternalInput").ap()
    consts_d = nc.dram_tensor("consts", [MC, NCONST], f32,
                              kind="ExternalInput").ap()

    out_d = nc.dram_tensor("out", [B, O, S_LOC], f32, kind="ExternalOutput").ap()
    aux_d = nc.dram_tensor("aux", [1, 1], f32, kind="ExternalOutput").ap()

    with tile.TileContext(nc, num_cores=NCORES) as tc:
      for _rep in range(repeat):
        with (
            tc.tile_pool(name="const", bufs=1) as cpool,
            tc.tile_pool(name="xp", bufs=1) as xpool,
            tc.tile_pool(name="scr", bufs=2) as spool,
            tc.tile_pool(name="dram", bufs=1, space="DRAM") as dpool,
        ):
            # ---- load x (f32r) + partial pooled sums on ACT ----
            POOL_CHUNK = 1024
            n_pc = S_LOC // POOL_CHUNK
            x_sb = []
            part_sb = cpool.tile([MC, B * n_pc], f32)
            const_sb = cpool.tile([MC, NCONST], f32)
            for b in range(B):
                xb = xpool.tile([MC, S_LOC], f32r, tag=f"x{b}")
                for j in range(n_pc):
                    sl = slice(j * POOL_CHUNK, (j + 1) * POOL_CHUNK)
                    nc.sync.dma_start(xb[:, sl], x_d[b, :, sl])
                    pscr = spool.tile([MC, POOL_CHUNK], f32, tag="pscr")
                    nc.scalar.activation(
                        pscr, xb[:, sl].bitcast(f32), AF.Copy,
                        bias=0.0, scale=1.0,
                        accum_out=part_sb[:, b * n_pc + j: b * n_pc + j + 1])
                x_sb.append(xb)

            nc.sync.dma_start(const_sb, consts_d)
            wct_sb = cpool.tile([MC, E * O], f32)
            nc.sync.dma_start(wct_sb, wct_d)
            wrt_sb = const_sb[:, _WRT]
            biasT_sb = const_sb[:, _BIAST]
            ones2_sb = const_sb[0:B, _ONES2]
            bsel_sb = const_sb[0:B, _BSEL]
            br2_sb = const_sb[0:B, _BR2]

            pooled_loc = cpool.tile([MC, B], f32)
            nc.vector.reduce_sum(
                pooled_loc,
                part_sb.rearrange("p (b j) -> p b j", b=B),
                axis=AX.X)

            # iotas for index extraction (identical rows on both partitions)
            iota_w = cpool.tile([B, E], f32)
            nc.gpsimd.iota(iota_w, pattern=[[O, E]], base=0,
                           channel_multiplier=0,
                           allow_small_or_imprecise_dtypes=True)
            if not fast:
                iota_b = cpool.tile([B, E], f32)
                nc.gpsimd.iota(iota_b, pattern=[[1, E]], base=0,
                               channel_multiplier=0,
                               allow_small_or_imprecise_dtypes=True)

            # ---- router: critical chain first, aux loss last ----
            with tc.tile_pool(name="rpsum", bufs=2, space="PSUM") as rpsum:
                # AllGather the partial sums in [rank*B, MC] row layout so
                # both bounce DMAs are contiguous; reduce over ranks with a
                # selector matmul.
                ident_sb = const_sb[:, _ID]
                selb_sb = const_sb[0:B * NCORES, _SELB]
                plt_t = rpsum.tile([MC, MC], f32, tag="t")
                plt_ps = plt_t[0:B, :]
                nc.tensor.transpose(plt_ps, pooled_loc, ident_sb)
                plt_sb = cpool.tile([B, MC], f32)
                nc.vector.tensor_copy(plt_sb, plt_ps)
                cc_in = dpool.tile([B, MC], f32)
                cc_out = dpool.tile([NCORES * B, MC], f32)
                nc.sync.dma_start(cc_in, plt_sb)
                nc.gpsimd.collective_compute(
                    "AllGather",
                    ALU.bypass,
                    replica_groups=[list(range(NCORES))],
                    ins=[cc_in.opt()],
                    outs=[cc_out.opt()],
                )
                ccg_sb = cpool.tile([NCORES * B, MC], f32)
                nc.sync.dma_start(ccg_sb, cc_out)
                pooled_t = rpsum.tile([MC, 8], f32, tag="r")
                pooled_ps = pooled_t[:, 0:B]
                nc.tensor.matmul(pooled_ps, lhsT=ccg_sb, rhs=selb_sb,
                                 start=True, stop=True)
                pooled_sb = cpool.tile([MC, B], f32)
                nc.vector.tensor_copy(pooled_sb, pooled_ps)

                ll_t = rpsum.tile([MC, 8], f32, tag="r")
                ll_ps = ll_t[0:B, 0:E]
                nc.tensor.matmul(ll_ps, lhsT=pooled_sb, rhs=wrt_sb,
                                 start=True, stop=True)
                logits = cpool.tile([B, E], f32)
                nc.vector.tensor_add(logits, ll_ps, br2_sb)

                mx = cpool.tile([B, 1], f32)
                nc.vector.reduce_max(mx, logits, axis=AX.X)
                mxn = cpool.tile([B, 1], f32)
                nc.vector.tensor_scalar_mul(mxn, mx, -1.0)
                expz = cpool.tile([B, E], f32)
                nc.scalar.activation(expz, logits, AF.Exp,
                                     bias=mxn[:, 0:1], scale=1.0)

                # top-2 masks + slot weights on UN-normalized exp values
                # (softmax normalization only matters for the aux loss)
                m1 = cpool.tile([B, 1], f32)
                nc.vector.reduce_max(m1, expz, axis=AX.X)
                mask1 = cpool.tile([B, E], f32)
                nc.vector.tensor_scalar(mask1, expz, m1[:, 0:1], None,
                                        op0=ALU.is_equal)
                pm = cpool.tile([B, E], f32)
                nc.vector.scalar_tensor_tensor(pm, mask1, -2.0, expz,
                                               op0=ALU.mult, op1=ALU.add)
                m2 = cpool.tile([B, 1], f32)
                nc.vector.reduce_max(m2, pm, axis=AX.X)
                mask2 = cpool.tile([B, E], f32)
                nc.vector.tensor_scalar(mask2, pm, m2[:, 0:1], None,
                                        op0=ALU.is_equal)

                # rt: [sw1 sw2 | i128_1 i128_2 (| ib_1 ib_2)]
                NRT = 4 if fast else 6
                rt = cpool.tile([B, NRT], f32)
                z2s = cpool.tile([B, 1], f32)
                nc.vector.tensor_add(z2s, m1, m2)
                rz = cpool.tile([B, 1], f32)
                nc.vector.reciprocal(rz, z2s)
                nc.vector.tensor_scalar(rt[:, 0:1], m1, rz[:, 0:1], None,
                                        op0=ALU.mult)
                nc.vector.tensor_scalar(rt[:, 1:2], m2, rz[:, 0:1], None,
                                        op0=ALU.mult)
                tmp = cpool.tile([B, E], f32, tag="tmp")
                nc.vector.tensor_mul(tmp, mask1, iota_w)
                nc.vector.reduce_sum(rt[:, 2:3], tmp, axis=AX.X)
                tmp2 = cpool.tile([B, E], f32, tag="tmp2")
                nc.vector.tensor_mul(tmp2, mask2, iota_w)
                nc.vector.reduce_sum(rt[:, 3:4], tmp2, axis=AX.X)
                if not fast:
                    tmp3 = cpool.tile([B, E], f32, tag="tmp3")
                    nc.vector.tensor_mul(tmp3, mask1, iota_b)
                    nc.vector.reduce_sum(rt[:, 4:5], tmp3, axis=AX.X)
                    tmp4 = cpool.tile([B, E], f32, tag="tmp4")
                    nc.vector.tensor_mul(tmp4, mask2, iota_b)
                    nc.vector.reduce_sum(rt[:, 5:6], tmp4, axis=AX.X)

                # broadcast rt across partitions; extract indices; select
                # weights via DVE dynamic SBUF slices, pre-scaled by slot
                # weight so the matmul output needs no scale
                DVE = mybir.EngineType.DVE
                bcs, biasP, wsel = [], [], []
                for b in range(B):
                    bc_t = rpsum.tile([MC, 8], f32, tag="r")
                    bc_ps = bc_t[:, 0:NRT]
                    nc.tensor.matmul(
                        bc_ps, lhsT=bsel_sb[:, b * MC:(b + 1) * MC],
                        rhs=rt, start=True, stop=True)
                    idxi = cpool.tile([1, NRT - 2], i32, tag=f"idxi{b}")
                    nc.vector.tensor_copy(idxi, bc_ps[0:1, 2:NRT])
                    bc = cpool.tile([MC, NRT], f32, tag=f"bc{b}")
                    nc.vector.tensor_copy(bc, bc_ps)
                    bcs.append(bc)
                    for k in range(TOPK):
                        vw = nc.values_load(idxi[0:1, k:k + 1], engines=[DVE],
                                            min_val=0, max_val=(E - 1) * O,
                                            skip_runtime_bounds_check=True)
                        ws = cpool.tile([MC, O], f32r, tag=f"ws{b}{k}")
                        nc.vector.tensor_scalar(
                            ws, wct_sb[:, bass.ds(vw, O)],
                            bc[:, k:k + 1], None, op0=ALU.mult)
                        wsel.append(ws)
                        if not fast:
                            vb = nc.values_load(idxi[0:1, 2 + k:3 + k],
                                                engines=[DVE],
                                                min_val=0, max_val=E - 1,
                                                skip_runtime_bounds_check=True)
                            bp = cpool.tile([MC, 1], f32, tag=f"bp{b}{k}")
                            nc.vector.tensor_scalar(
                                bp, biasT_sb[:, bass.ds(vb, 1)],
                                bc[:, k:k + 1], None, op0=ALU.mult)
                            biasP.append(bp)

                # aux loss = E/B^2 * sum_e (sum_b mask1)(sum_b probs)
                sm = cpool.tile([B, 1], f32)
                nc.vector.reduce_sum(sm, expz, axis=AX.X)
                rs = cpool.tile([B, 1], f32)
                nc.vector.reciprocal(rs, sm)
                probs = cpool.tile([B, E], f32)
                nc.vector.tensor_scalar(probs, expz, rs[:, 0:1], None,
                                        op0=ALU.mult)
                fs_t = rpsum.tile([MC, 8], f32, tag="r")
                fs_ps = fs_t[0:1, 0:E]
                nc.tensor.matmul(fs_ps, lhsT=ones2_sb, rhs=mask1,
                                 start=True, stop=True)
                fs_sb = cpool.tile([1, E], f32)
                nc.vector.tensor_copy(fs_sb, fs_ps)
                ps_t = rpsum.tile([MC, 8], f32, tag="r")
                ps_ps = ps_t[0:1, 0:E]
                nc.tensor.matmul(ps_ps, lhsT=ones2_sb, rhs=probs,
                                 start=True, stop=True)
                fp = cpool.tile([1, E], f32)
                nc.vector.tensor_mul(fp, fs_sb, ps_ps)
                aux1 = cpool.tile([1, 1], f32)
                nc.vector.reduce_sum(aux1, fp, axis=AX.X)
                aux2 = cpool.tile([1, 1], f32)
                nc.vector.tensor_scalar_mul(aux2, aux1, float(E) / (B * B))
                nc.gpsimd.dma_start(aux_d, aux2)

            # ---- selected-expert GEMMs + fused epilogue ----
            with (
                tc.tile_pool(name="gpsum", bufs=4, space="PSUM") as gpsum,
                tc.tile_pool(name="ep", bufs=6) as epool,
                tc.tile_pool(name="op", bufs=4) as opool,
            ):
                for b in range(B):
                    w0, w1 = wsel[2 * b], wsel[2 * b + 1]
                    sw0 = bcs[b][:, 0:1]
                    sw1 = bcs[b][:, 1:2]
                    for jj in range(N_TILES):
                        rhs = x_sb[b][:, jj * NT:(jj + 1) * NT]
                        z0 = gpsum.tile([O, NT], f32, tag="z")
                        z1 = gpsum.tile([O, NT], f32, tag="z")
                        for h in range(NT // 512):
                            hs = slice(h * 512, (h + 1) * 512)
                            nc.tensor.matmul(z0[:, hs], lhsT=w0,
                                             rhs=rhs[:, hs],
                                             start=True, stop=True)
                        for h in range(NT // 512):
                            hs = slice(h * 512, (h + 1) * 512)
                            nc.tensor.matmul(z1[:, hs], lhsT=w1,
                                             rhs=rhs[:, hs],
                                             start=True, stop=True)
                        # weights already carry the slot scales sw0/sw1
                        ot = opool.tile([O, NT], f32, tag="o")
                        if fast:
                            y0 = epool.tile([O, NT], f32, tag="y0")
                            nc.scalar.activation(y0, z0, AF.Relu,
                                                 bias=0.0, scale=1.0)
                            nc.vector.scalar_tensor_tensor(
                                ot, z1, 0.0, y0, op0=ALU.max, op1=ALU.add)
                        else:
                            b0 = biasP[2 * b]
                            b1 = biasP[2 * b + 1]
                            mode = MODES[jj % len(MODES)]
                            y0 = epool.tile([O, NT], f32, tag="y0")
                            nc.scalar.activation(y0, z0, AF.Relu,
                                                 bias=b0[:, 0:1], scale=1.0)
                            ts = epool.tile([O, NT], f32, tag="ts")
                            if mode == "A":
                                nc.vector.tensor_scalar(
                                    ts, z1, b1[:, 0:1], None, op0=ALU.add)
                            else:
                                nc.scalar.activation(ts, z1, AF.Identity,
                                                     bias=b1[:, 0:1],
                                                     scale=1.0)
                            nc.vector.scalar_tensor_tensor(
                                ot, ts, 0.0, y0, op0=ALU.max, op1=ALU.add)
                        nc.sync.dma_start(
                            out_d[b, :, jj * NT:(jj + 1) * NT], ot)

    nc.compile()
    return nc


def _get_nc(fast: bool, repeat: int = 1):
    key = ("nc", fast, repeat)
    if key not in _BUILD_CACHE:
        _BUILD_CACHE[key] = _build(fast, repeat)
    return _BUILD_CACHE[key]


def _pack_consts(Wc, gamma, beta, rmean, rvar, Wr, br):
    inv = gamma / np.sqrt(rvar + BN_EPS)                     # [E, O]
    w_eff = Wc * inv[:, :, None]                             # [E, O, MC]
    wct = np.ascontiguousarray(
        w_eff.transpose(2, 0, 1).reshape(MC, E * O))
    bias_eff = (beta - rmean * inv).T                        # [O, E]
    consts = np.zeros((MC, NCONST), np.float32)
    consts[:, _WRT] = Wr.T / np.float32(S)                   # /S folds GAP mean
    consts[:, _BIAST] = bias_eff
    consts[0:B, _ONES2] = 1.0
    bsel = np.zeros((B, B * MC), np.float32)
    for b in range(B):
        bsel[b, b * MC:(b + 1) * MC] = 1.0
    consts[0:B, _BSEL] = bsel
    consts[0:B, _BR2] = np.broadcast_to(br, (B, E))
    consts[:, _ID] = np.eye(MC, dtype=np.float32)
    rows = np.arange(B * NCORES)
    for bb in range(B):
        consts[rows[rows % B == bb], _SELB.start + bb] = 1.0
    return wct, consts, bias_eff


def kernel(f0, f1, f2, f3, Wc, gamma, beta, rmean, rvar, Wr, br, top_k):
    global LAST_RESULTS
    from concourse.bass_utils import run_bass_kernel_spmd

    assert int(top_k) == TOPK
    f = [np.asarray(t, np.float32) for t in (f0, f1, f2, f3)]
    wct, consts, bias_eff = _pack_consts(
        np.asarray(Wc, np.float32), np.asarray(gamma, np.float32),
        np.asarray(beta, np.float32), np.asarray(rmean, np.float32),
        np.asarray(rvar, np.float32), np.asarray(Wr, np.float32),
        np.asarray(br, np.float32))
    fast = bool(np.all(bias_eff == 0.0))

    x = np.concatenate(f, axis=1).reshape(B, MC, S)
    in_maps = []
    for c in range(NCORES):
        in_maps.append(dict(
            consts=consts,
            wct=wct,
            x=np.ascontiguousarray(x[:, :, c * S_LOC:(c + 1) * S_LOC]),
        ))

    nc = _get_nc(fast)
    res = run_bass_kernel_spmd(nc, in_maps, core_ids=list(range(NCORES)),
                               trace=TRACE, **TRACE_KWARGS)
    LAST_RESULTS = res

    out = np.empty((B, O, S), np.float32)
    for c in range(NCORES):
        out[:, :, c * S_LOC:(c + 1) * S_LOC] = res.results[c]["out"]
    aux = np.float32(res.results[0]["aux"][0, 0])
    return out.reshape(B, O, D, H, W), aux
